# revision 9
# baseline (speedup 1.0000x reference)
"""GroupedAttention Trainium2 kernel (8 NeuronCores, SPMD, no collectives).

Problem: x[2,2048,1024] -> grouped qkv (G=8 block-diag) -> 16-head attention
-> grouped proj (G=8 block-diag) + bias.

Sharding: core c owns heads (2c, 2c+1) and proj group c. The proj group c
consumes exactly the attention outputs of heads 2c/2c+1 and produces output
channels [128c, 128c+128) -- so each core computes an independent channel
slice of the final output; outputs are concatenated on the host.

The qkv grouping does NOT align with heads (each qkv group emits a mixed
384-channel slice), so per core we hand it the three 128-channel x-slices
(for its q, k and v blocks) pre-transposed to channel-major [128, B*N],
plus the matching [128(in),128(out)] weight blocks.

Pipeline layout per core:
  phase 1: qT/kT ([128,2048] per batch, rows 0:64 head0 / 64:128 head1) and
    vaug ([128(m), 16(mt), 65] per (b,h); col 64 = ones for the softmax
    denominator) via f32r matmuls; PSUM evacuation split between ACT/DVE.
  phase 2 (per (b, nb) round over 512-query windows): 16 m-tile steps of
    scores (PE) -> exp (ACT, or Schraudolph bit-trick on DVE for a few
    m-tiles to balance engines) -> accumulating AV matmul (PE). Softmax
    denominator rides as row 64 of the AV output. Normalize = reciprocal +
    partition-broadcast (Pool) + multiply into persistent stk.
  phase 3: proj tiles interleaved one round behind, bias-add, per-tile DMA.
"""

import numpy as np
from contextlib import ExitStack

import concourse.bass as bass
import concourse.tile as tile
from concourse import bacc, mybir
from concourse.bass_utils import run_bass_kernel_spmd

F32 = mybir.dt.float32
F32R = mybir.dt.float32r
I32 = mybir.dt.int32
EXP = mybir.ActivationFunctionType.Exp
COPY = mybir.ActivationFunctionType.Copy
MULT = mybir.AluOpType.mult
ADD = mybir.AluOpType.add

B = 2
N = 2048
C = 1024
H = 16
G = 8
D = 64          # head dim
BN = B * N      # 4096
W = 512         # attention n-window per round
NB = N // W     # rounds per batch = 4
MT = N // 128   # m-tiles per batch = 16
SCALE = D ** -0.5

# m-tiles per round whose exp runs on DVE via the Schraudolph bit trick
# (exp(x) ~= bitcast_f32(int32(x*2^23/ln2 + B))), to offload the ACT engine.
DVE_MTS = (6, 9, 12)
SCHRAU_A = SCALE * (2.0 ** 23) / np.log(2.0)          # applied to raw scores
SCHRAU_B = (127.0 - 0.0219) * (2.0 ** 23) + 0.5       # centered + trunc->round

_CACHE = {}


def _r(ap):
    return ap if ap.dtype == F32R else ap.bitcast(F32R)


def _build_nc():
    nc = bacc.Bacc("TRN2", target_bir_lowering=False, debug=False, num_devices=8)

    xq = nc.dram_tensor("xq", [128, BN], F32, kind="ExternalInput").ap()
    xk = nc.dram_tensor("xk", [128, BN], F32, kind="ExternalInput").ap()
    xv = nc.dram_tensor("xv", [128, BN], F32, kind="ExternalInput").ap()
    wq = nc.dram_tensor("wq", [128, 128], F32, kind="ExternalInput").ap()
    wk = nc.dram_tensor("wk", [128, 128], F32, kind="ExternalInput").ap()
    wv = nc.dram_tensor("wv", [128, 256], F32, kind="ExternalInput").ap()
    wp0 = nc.dram_tensor("wp0", [64, 256], F32, kind="ExternalInput").ap()
    wp1 = nc.dram_tensor("wp1", [64, 256], F32, kind="ExternalInput").ap()
    bias = nc.dram_tensor("bias", [128, 128], F32, kind="ExternalInput").ap()
    y = nc.dram_tensor("y", [B, N, 128], F32, kind="ExternalOutput").ap()

    with ExitStack() as ctx:
        tc = ctx.enter_context(tile.TileContext(nc))
        nc_ = tc.nc

        persist = ctx.enter_context(tc.tile_pool(name="persist", bufs=1))

        # ---- input DMAs: split across SP and Pool queues, ordered by
        # consumption (qk of b0 first, then v of b0, then b1). ----
        wq_t = persist.tile([128, 128], F32R, tag="wq")
        wk_t = persist.tile([128, 128], F32R, tag="wk")
        wv_t = persist.tile([128, 256], F32R, tag="wv")
        wp0_t = persist.tile([64, 256], F32R, tag="wp0")
        wp1_t = persist.tile([64, 256], F32R, tag="wp1")
        bias_t = persist.tile([128, 128], F32, tag="bias")
        xq_t = persist.tile([128, BN], F32R, tag="xq")
        xk_t = persist.tile([128, BN], F32R, tag="xk")
        xv_t = persist.tile([128, BN], F32R, tag="xv")

        # f32 -> f32r casting DMAs must go through the gpsimd (SWDGE) queue.
        def chunk(dst, src, i):
            s = slice(i * 1024, (i + 1) * 1024)
            nc_.gpsimd.dma_start(out=dst[:, s], in_=src[:, s])

        nc_.gpsimd.dma_start(out=wq_t, in_=wq)
        nc_.gpsimd.dma_start(out=wk_t, in_=wk)
        for i in range(2):
            chunk(xq_t, xq, i)
            chunk(xk_t, xk, i)
        nc_.gpsimd.dma_start(out=wv_t, in_=wv)
        for i in range(2):
            chunk(xv_t, xv, i)
        for i in range(2, 4):
            chunk(xq_t, xq, i)
            chunk(xk_t, xk, i)
        for i in range(2, 4):
            chunk(xv_t, xv, i)
        nc_.gpsimd.dma_start(out=wp0_t, in_=wp0)
        nc_.gpsimd.dma_start(out=wp1_t, in_=wp1)
        nc_.sync.dma_start(out=bias_t, in_=bias)

        # ---- persistent activations ----
        qT = [persist.tile([128, N], F32R, tag=f"qT{b}", name=f"qT{b}")
              for b in range(B)]   # rows 0:64 h0, 64:128 h1
        kT = [persist.tile([128, N], F32R, tag=f"kT{b}", name=f"kT{b}")
              for b in range(B)]
        # vaug[b*2+h]: [128(m), MT, 65]; col 64 = ones (softmax denominator)
        vaug = [persist.tile([128, MT, 65], F32R, tag=f"vaug{i}", name=f"vaug{i}")
                for i in range(4)]
        ones_f = persist.tile([128, MT, 1], F32, tag="ones_f")
        nc_.gpsimd.memset(ones_f, 1.0)
        for t in vaug:
            nc_.vector.tensor_copy(out=t[:, :, 64:65], in_=ones_f)
        # normalized attention outputs per (b, h): [64(d), N]
        stk = [[persist.tile([64, N], F32R, tag=f"stk{b}{h}", name=f"stk{b}{h}")
                for h in range(2)]
               for b in range(B)]

        # ---- phase 2 + 3 pools (phase 1 borrows the stp ring) ----
        stp = ctx.enter_context(tc.tile_pool(name="stp", bufs=2, space="PSUM"))
        avp = ctx.enter_context(tc.tile_pool(name="avp", bufs=4, space="PSUM"))
        ptp = ctx.enter_context(tc.tile_pool(name="ptp", bufs=4))
        nrm = ctx.enter_context(tc.tile_pool(name="nrm", bufs=2))
        outp = ctx.enter_context(tc.tile_pool(name="outp", bufs=8))

        def ph1_qk(b, i):
            # one 512-column window of qT/kT for batch b
            s = slice(b * N + i * 512, b * N + (i + 1) * 512)
            sl = slice(i * 512, (i + 1) * 512)
            pq = stp.tile([128, 2 * W], F32, tag="st", name=f"pq{b}{i}")
            nc_.tensor.matmul(pq[:, 0:512], _r(wq_t), _r(xq_t[:, s]),
                              start=True, stop=True)
            nc_.scalar.activation(out=qT[b][:, sl], in_=pq[:, 0:512], func=COPY)
            pk = stp.tile([128, 2 * W], F32, tag="st", name=f"pk{b}{i}")
            nc_.tensor.matmul(pk[:, 0:512], _r(wk_t), _r(xk_t[:, s]),
                              start=True, stop=True)
            nc_.vector.tensor_copy(out=kT[b][:, sl], in_=pk[:, 0:512])

        def ph1_v(b, g):
            # one 128-row m-tile of v for batch b
            s = slice(b * N + g * 128, b * N + (g + 1) * 128)
            pv = stp.tile([128, 2 * W], F32, tag="st", name=f"pv{b}{g}")
            nc_.tensor.matmul(pv[:, 0:256], _r(xv_t[:, s]), _r(wv_t),
                              start=True, stop=True)
            nc_.scalar.activation(out=vaug[b * 2][:, g, 0:64], in_=pv[:, 0:64],
                                  func=COPY)
            nc_.vector.tensor_copy(out=vaug[b * 2 + 1][:, g, 0:64],
                                   in_=pv[:, 64:128])

        for i in range(4):
            ph1_qk(0, i)
        for g in range(MT):
            ph1_v(0, g)

        # remaining phase-1 pieces for b=1, interleaved into b=0's rounds
        ph1_rest = ([lambda i=i: ph1_qk(1, i) for i in range(4)]
                    + [lambda g=g: ph1_v(1, g) for g in range(MT)])

        def round_(b, nb, extra=()):
            n0 = nb * W
            extra = list(extra)
            av = [avp.tile([128, W], F32, tag="av", name=f"av{b}{nb}{h}")
                  for h in range(2)]

            def emit_av(mt, pt):
                for h in range(2):
                    nc_.tensor.matmul(
                        av[h][0:65, :],
                        _r(vaug[b * 2 + h][:, mt, :]),
                        _r(pt[:, h * W:(h + 1) * W]),
                        start=(mt == 0), stop=(mt == MT - 1),
                    )

            prev = None
            for mt in range(MT):
                m0 = mt * 128
                st = stp.tile([128, 2 * W], F32, tag="st", name=f"st{b}{nb}{mt}")
                for h in range(2):
                    hs = slice(h * 64, (h + 1) * 64)
                    nc_.tensor.matmul(
                        st[:, h * W:(h + 1) * W],
                        _r(kT[b][hs, m0:m0 + 128]),
                        _r(qT[b][hs, n0:n0 + W]),
                        start=True, stop=True,
                    )
                if prev is not None:
                    emit_av(*prev)
                if extra:
                    extra.pop(0)()
                pt = ptp.tile([128, 2 * W], F32R, tag="pt",
                              name=f"pt{b}{nb}{mt}")
                if mt in DVE_MTS:
                    # Schraudolph: int32 bits of exp(s*SCALE), then a
                    # bit-exact same-dtype copy into the f32r tile.
                    pi = ptp.tile([128, 2 * W], I32, tag="pti",
                                  name=f"pi{b}{nb}{mt}", bufs=2)
                    nc_.vector.tensor_scalar(
                        out=pi, in0=st,
                        scalar1=float(SCHRAU_A), scalar2=float(SCHRAU_B),
                        op0=MULT, op1=ADD,
                    )
                    nc_.vector.tensor_copy(out=pt, in_=pi.bitcast(F32R))
                else:
                    nc_.scalar.activation(out=pt, in_=st, func=EXP, scale=SCALE)
                prev = (mt, pt)
            emit_av(*prev)
            for f in extra:
                f()

            # normalize: row 64 of av holds Z[n]; rbz = 1/Z broadcast to the
            # 64 head dims, multiply into persistent stk (off critical path).
            for h in range(2):
                zr = nrm.tile([1, W], F32, tag="zr", name=f"zr{b}{nb}{h}")
                nc_.vector.tensor_copy(out=zr, in_=av[h][64:65, :])
                rz = nrm.tile([1, W], F32, tag="rz", name=f"rz{b}{nb}{h}")
                nc_.vector.reciprocal_approx_fast(out=rz, in_=zr)
                rb = nrm.tile([64, W], F32, tag="rb", name=f"rb{b}{nb}{h}")
                nc_.gpsimd.partition_broadcast(rb, rz)
                nc_.vector.tensor_mul(
                    stk[b][h][:, n0:n0 + W], av[h][0:64, :], rb)

        def proj(b, nb):
            for nt in range(nb * 4, nb * 4 + 4):
                s = slice(nt * 128, (nt + 1) * 128)
                pp = stp.tile([128, 2 * W], F32, tag="st", name=f"pp{b}{nt}")
                nc_.tensor.matmul(pp[:, 0:256], _r(stk[b][0][:, s]), _r(wp0_t),
                                  start=True, stop=False)
                nc_.tensor.matmul(pp[:, 0:256], _r(stk[b][1][:, s]), _r(wp1_t),
                                  start=False, stop=True)
                ot = outp.tile([128, 128], F32, tag="ot", name=f"ot{b}{nt}")
                nc_.vector.tensor_add(ot, pp[:, 0:128], bias_t)
                nc_.sync.dma_start(out=y[b, s, :], in_=ot)

        rounds = [(b, nb) for b in range(B) for nb in range(NB)]
        for i, (b, nb) in enumerate(rounds):
            if b == 0:
                lo = nb * 5
                extra = ph1_rest[lo:lo + 5]
            else:
                extra = []
            round_(b, nb, extra)
            if i >= 1:
                proj(*rounds[i - 1])
        proj(*rounds[-1])

    nc.finalize()
    return nc


def _core_inputs(x, w_qkv, w_proj, b_proj, c):
    h0 = 2 * c
    gq, oq = divmod(64 * h0, 384)
    gk, ok = divmod(C + 64 * h0, 384)
    gv, ov = divmod(2 * C + 64 * h0, 384)

    def xsl(g):
        # [B,N,128] slice -> channel-major [128, B*N]
        return np.ascontiguousarray(
            x[:, :, 128 * g:128 * (g + 1)].reshape(BN, 128).T
        )

    wv = np.zeros((128, 256), np.float32)
    wv[:, 0:128] = w_qkv[gv][:, ov:ov + 128]
    wp = w_proj[c]
    wp0 = np.zeros((64, 256), np.float32)
    wp0[:, 0:128] = wp[0:64, :]
    wp1 = np.zeros((64, 256), np.float32)
    wp1[:, 0:128] = wp[64:128, :]
    return {
        "xq": xsl(gq),
        "xk": xsl(gk),
        "xv": xsl(gv),
        "wq": np.ascontiguousarray(w_qkv[gq][:, oq:oq + 128]),
        "wk": np.ascontiguousarray(w_qkv[gk][:, ok:ok + 128]),
        "wv": wv,
        "wp0": wp0,
        "wp1": wp1,
        "bias": np.ascontiguousarray(
            np.broadcast_to(b_proj[128 * c:128 * (c + 1)], (128, 128))
        ).astype(np.float32),
    }


def kernel(x, w_qkv, w_proj, b_proj, _trace=False, _trace_kwargs=None):
    x = np.asarray(x, np.float32)
    w_qkv = np.asarray(w_qkv, np.float32)
    w_proj = np.asarray(w_proj, np.float32)
    b_proj = np.asarray(b_proj, np.float32)

    if "nc" not in _CACHE:
        _CACHE["nc"] = _build_nc()
    nc = _CACHE["nc"]

    in_maps = [_core_inputs(x, w_qkv, w_proj, b_proj, c) for c in range(8)]
    res = run_bass_kernel_spmd(
        nc, in_maps, list(range(8)),
        trace=_trace, **(_trace_kwargs or {}),
    )
    out = np.concatenate([res.results[c]["y"] for c in range(8)], axis=2)
    if _trace:
        return out, res
    return out


# revision 14
# speedup vs baseline: 1.1784x; 1.1784x over previous
"""GroupedAttention Trainium2 kernel (8 NeuronCores, SPMD, no collectives).

Problem: x[2,2048,1024] -> grouped qkv (G=8 block-diag) -> 16-head attention
-> grouped proj (G=8 block-diag) + bias.

Sharding: core c owns heads (2c, 2c+1) and proj group c. The proj group c
consumes exactly the attention outputs of heads 2c/2c+1 and produces output
channels [128c, 128c+128) -- so each core computes an independent channel
slice of the final output; outputs are concatenated on the host.

The qkv grouping does NOT align with heads (each qkv group emits a mixed
384-channel slice), so per core we hand it the three 128-channel x-slices
(for its q, k and v blocks) pre-transposed to channel-major [128, B*N],
plus the matching [128(in),128(out)] weight blocks.

Pipeline layout per core:
  phase 1: qT/kT ([128,2048] per batch, rows 0:64 head0 / 64:128 head1) and
    vaug ([128(m), 16(mt), 65] per (b,h); col 64 = ones for the softmax
    denominator) via f32r matmuls; PSUM evacuation split between ACT/DVE.
  phase 2 (per (b, nb) round over 512-query windows): 16 m-tile steps of
    scores (PE) -> exp (ACT, or Schraudolph bit-trick on DVE for a few
    m-tiles to balance engines) -> accumulating AV matmul (PE). Softmax
    denominator rides as row 64 of the AV output. Normalize = reciprocal +
    partition-broadcast (Pool) + multiply into persistent stk.
  phase 3: proj tiles interleaved one round behind, bias-add, per-tile DMA.
"""

import numpy as np
from contextlib import ExitStack

import concourse.bass as bass
import concourse.tile as tile
from concourse import bacc, mybir
from concourse.bass_utils import run_bass_kernel_spmd

F32 = mybir.dt.float32
F32R = mybir.dt.float32r
I32 = mybir.dt.int32
EXP = mybir.ActivationFunctionType.Exp
COPY = mybir.ActivationFunctionType.Copy
MULT = mybir.AluOpType.mult
ADD = mybir.AluOpType.add

B = 2
N = 2048
C = 1024
H = 16
G = 8
D = 64          # head dim
BN = B * N      # 4096
W = 512         # attention n-window per round
NB = N // W     # rounds per batch = 4
MT = N // 128   # m-tiles per batch = 16
SCALE = D ** -0.5

# m-tiles per round whose exp runs on DVE via the Schraudolph bit trick
# (exp(x) ~= bitcast_f32(int32(x*2^23/ln2 + B))), to offload the ACT engine.
DVE_MTS = (6, 9, 12)
SCHRAU_A = SCALE * (2.0 ** 23) / np.log(2.0)          # applied to raw scores
SCHRAU_B = (127.0 - 0.057) * (2.0 ** 23) + 0.5       # centered + trunc->round

_CACHE = {}


def _r(ap):
    return ap if ap.dtype == F32R else ap.bitcast(F32R)


def _build_nc():
    nc = bacc.Bacc("TRN2", target_bir_lowering=False, debug=False, num_devices=8)

    xq = nc.dram_tensor("xq", [128, BN], F32, kind="ExternalInput").ap()
    xk = nc.dram_tensor("xk", [128, BN], F32, kind="ExternalInput").ap()
    xv = nc.dram_tensor("xv", [128, BN], F32, kind="ExternalInput").ap()
    wq = nc.dram_tensor("wq", [128, 128], F32, kind="ExternalInput").ap()
    wk = nc.dram_tensor("wk", [128, 128], F32, kind="ExternalInput").ap()
    wv = nc.dram_tensor("wv", [128, 256], F32, kind="ExternalInput").ap()
    wp0 = nc.dram_tensor("wp0", [65, 256], F32, kind="ExternalInput").ap()
    wp1 = nc.dram_tensor("wp1", [65, 256], F32, kind="ExternalInput").ap()
    y = nc.dram_tensor("y", [B, N, 128], F32, kind="ExternalOutput").ap()

    with ExitStack() as ctx:
        tc = ctx.enter_context(tile.TileContext(nc))
        nc_ = tc.nc

        persist = ctx.enter_context(tc.tile_pool(name="persist", bufs=1))

        # ---- input DMAs: split across SP and Pool queues, ordered by
        # consumption (qk of b0 first, then v of b0, then b1). ----
        wq_t = persist.tile([128, 128], F32R, tag="wq")
        wk_t = persist.tile([128, 128], F32R, tag="wk")
        wv_t = persist.tile([128, 256], F32R, tag="wv")
        wp0_t = persist.tile([65, 256], F32R, tag="wp0")
        wp1_t = persist.tile([65, 256], F32R, tag="wp1")
        xq_t = persist.tile([128, BN], F32R, tag="xq")
        xk_t = persist.tile([128, BN], F32R, tag="xk")
        xv_t = persist.tile([128, BN], F32R, tag="xv")

        # f32 -> f32r casting DMAs must go through the gpsimd (SWDGE) queue.
        def chunk(dst, src, i):
            s = slice(i * 1024, (i + 1) * 1024)
            nc_.gpsimd.dma_start(out=dst[:, s], in_=src[:, s])

        nc_.gpsimd.dma_start(out=wq_t, in_=wq)
        nc_.gpsimd.dma_start(out=wk_t, in_=wk)
        for i in range(2):
            chunk(xq_t, xq, i)
            chunk(xk_t, xk, i)
        nc_.gpsimd.dma_start(out=wv_t, in_=wv)
        for i in range(2):
            chunk(xv_t, xv, i)
        for i in range(2, 4):
            chunk(xq_t, xq, i)
            chunk(xk_t, xk, i)
        for i in range(2, 4):
            chunk(xv_t, xv, i)
        nc_.gpsimd.dma_start(out=wp0_t, in_=wp0)
        nc_.gpsimd.dma_start(out=wp1_t, in_=wp1)

        # ---- persistent activations ----
        qT = [persist.tile([128, N], F32R, tag=f"qT{b}", name=f"qT{b}")
              for b in range(B)]   # rows 0:64 h0, 64:128 h1
        kT = [persist.tile([128, N], F32R, tag=f"kT{b}", name=f"kT{b}")
              for b in range(B)]
        # vaug[b*2+h]: [128(m), MT, 65]; col 64 = ones (softmax denominator)
        vaug = [persist.tile([128, MT, 65], F32R, tag=f"vaug{i}", name=f"vaug{i}")
                for i in range(4)]
        ones_f = persist.tile([128, MT, 1], F32, tag="ones_f")
        nc_.gpsimd.memset(ones_f, 1.0)
        for t in vaug:
            nc_.vector.tensor_copy(out=t[:, :, 64:65], in_=ones_f)
        # normalized attention outputs per (b, h): [65(d), N]; row 64 is
        # ones so the proj matmul's 65-deep contraction adds the bias row
        # carried in wp0.
        stk = [[persist.tile([65, N], F32R, tag=f"stk{b}{h}", name=f"stk{b}{h}")
                for h in range(2)]
               for b in range(B)]
        ones_row = persist.tile([65, N], F32, tag="ones_row")
        nc_.gpsimd.memset(ones_row[64:65, :], 1.0)
        for b in range(B):
            for h in range(2):
                nc_.vector.tensor_copy(out=stk[b][h][64:65, :],
                                       in_=ones_row[64:65, :])

        # ---- phase 2 + 3 pools (phase 1 borrows the stp ring) ----
        stp = ctx.enter_context(tc.tile_pool(name="stp", bufs=2, space="PSUM"))
        avp = ctx.enter_context(tc.tile_pool(name="avp", bufs=4, space="PSUM"))
        ptp = ctx.enter_context(tc.tile_pool(name="ptp", bufs=4))
        nrm = ctx.enter_context(tc.tile_pool(name="nrm", bufs=2))
        outp = ctx.enter_context(tc.tile_pool(name="outp", bufs=8))

        def ph1_qk(b, i):
            # one 512-column window of qT/kT for batch b
            s = slice(b * N + i * 512, b * N + (i + 1) * 512)
            sl = slice(i * 512, (i + 1) * 512)
            pq = avp.tile([128, W], F32, tag="av", name=f"pq{b}{i}")
            nc_.tensor.matmul(pq, _r(wq_t), _r(xq_t[:, s]), start=True, stop=True)
            nc_.scalar.activation(out=qT[b][:, sl], in_=pq, func=COPY)
            pk = avp.tile([128, W], F32, tag="av", name=f"pk{b}{i}")
            nc_.tensor.matmul(pk, _r(wk_t), _r(xk_t[:, s]), start=True, stop=True)
            nc_.vector.tensor_copy(out=kT[b][:, sl], in_=pk)

        def ph1_v(b, g):
            # one 128-row m-tile of v for batch b
            s = slice(b * N + g * 128, b * N + (g + 1) * 128)
            pv = avp.tile([128, W], F32, tag="av", name=f"pv{b}{g}")
            nc_.tensor.matmul(pv[:, 0:256], _r(xv_t[:, s]), _r(wv_t),
                              start=True, stop=True)
            nc_.scalar.activation(out=vaug[b * 2][:, g, 0:64], in_=pv[:, 0:64],
                                  func=COPY)
            nc_.vector.tensor_copy(out=vaug[b * 2 + 1][:, g, 0:64],
                                   in_=pv[:, 64:128])

        for i in range(4):
            ph1_qk(0, i)
        for g in range(MT):
            ph1_v(0, g)

        # remaining phase-1 pieces for b=1, interleaved into b=0's rounds
        ph1_rest = ([lambda i=i: ph1_qk(1, i) for i in range(4)]
                    + [lambda g=g: ph1_v(1, g) for g in range(MT)])

        def round_(b, nb, extra=()):
            n0 = nb * W
            extra = list(extra)
            av = [avp.tile([128, W], F32, tag="av", name=f"av{b}{nb}{h}")
                  for h in range(2)]

            def emit_av(mt, pt):
                for h in range(2):
                    nc_.tensor.matmul(
                        av[h][0:65, :],
                        _r(vaug[b * 2 + h][:, mt, :]),
                        _r(pt[:, h * W:(h + 1) * W]),
                        start=(mt == 0), stop=(mt == MT - 1),
                    )

            prev = None
            for mt in range(MT):
                m0 = mt * 128
                st = stp.tile([128, 2 * W], F32, tag="st", name=f"st{b}{nb}{mt}")
                for h in range(2):
                    hs = slice(h * 64, (h + 1) * 64)
                    nc_.tensor.matmul(
                        st[:, h * W:(h + 1) * W],
                        _r(kT[b][hs, m0:m0 + 128]),
                        _r(qT[b][hs, n0:n0 + W]),
                        start=True, stop=True,
                    )
                if prev is not None:
                    emit_av(*prev)
                if extra:
                    extra.pop(0)()
                pt = ptp.tile([128, 2 * W], F32R, tag="pt",
                              name=f"pt{b}{nb}{mt}")
                if mt in DVE_MTS:
                    # Schraudolph: int32 bits of exp(s*SCALE), then a
                    # bit-exact same-dtype copy into the f32r tile.
                    pi = ptp.tile([128, 2 * W], I32, tag="pti",
                                  name=f"pi{b}{nb}{mt}", bufs=2)
                    nc_.vector.tensor_scalar(
                        out=pi, in0=st,
                        scalar1=float(SCHRAU_A), scalar2=float(SCHRAU_B),
                        op0=MULT, op1=ADD,
                    )
                    nc_.vector.tensor_copy(out=pt, in_=pi.bitcast(F32R))
                else:
                    nc_.scalar.activation(out=pt, in_=st, func=EXP, scale=SCALE)
                prev = (mt, pt)
            emit_av(*prev)
            for f in extra:
                f()

            def normalize():
                # row 64 of av holds Z[n]; rbz = 1/Z broadcast to the 64 head
                # dims, multiply into persistent stk (off critical path).
                for h in range(2):
                    zr = nrm.tile([1, W], F32, tag="zr", name=f"zr{b}{nb}{h}")
                    nc_.vector.tensor_copy(out=zr, in_=av[h][64:65, :])
                    rz = nrm.tile([1, W], F32, tag="rz", name=f"rz{b}{nb}{h}")
                    nc_.vector.reciprocal_approx_fast(out=rz, in_=zr)
                    rb = nrm.tile([64, W], F32, tag="rb", name=f"rb{b}{nb}{h}")
                    nc_.gpsimd.partition_broadcast(rb, rz)
                    nc_.vector.tensor_mul(
                        stk[b][h][0:64, n0:n0 + W], av[h][0:64, :], rb)
            return normalize

        def proj(b, nb):
            for nt in range(nb * 4, nb * 4 + 4):
                s = slice(nt * 128, (nt + 1) * 128)
                pp = stp.tile([128, 2 * W], F32, tag="st", name=f"pp{b}{nt}")
                nc_.tensor.matmul(pp[:, 0:256], _r(stk[b][0][:, s]), _r(wp0_t),
                                  start=True, stop=False)
                nc_.tensor.matmul(pp[:, 0:256], _r(stk[b][1][:, s]), _r(wp1_t),
                                  start=False, stop=True)
                ot = outp.tile([128, 128], F32, tag="ot", name=f"ot{b}{nt}")
                nc_.vector.tensor_copy(out=ot, in_=pp[:, 0:128])
                nc_.sync.dma_start(out=y[b, s, :], in_=ot)

        rounds = [(b, nb) for b in range(B) for nb in range(NB)]
        for i, (b, nb) in enumerate(rounds):
            if b == 0:
                lo = nb * 5
                extra = ph1_rest[lo:lo + 5]
            else:
                extra = []
            normalize = round_(b, nb, extra)
            # proj of the previous round goes first so its PSUM evacuation
            # copies run on DVE before the (lazy) normalize of this round.
            if i >= 1:
                proj(*rounds[i - 1])
            normalize()
        proj(*rounds[-1])

    nc.finalize()
    return nc


def _core_inputs(x, w_qkv, w_proj, b_proj, c):
    h0 = 2 * c
    gq, oq = divmod(64 * h0, 384)
    gk, ok = divmod(C + 64 * h0, 384)
    gv, ov = divmod(2 * C + 64 * h0, 384)

    def xsl(g):
        # [B,N,128] slice -> channel-major [128, B*N]
        return np.ascontiguousarray(
            x[:, :, 128 * g:128 * (g + 1)].reshape(BN, 128).T
        )

    wv = np.zeros((128, 256), np.float32)
    wv[:, 0:128] = w_qkv[gv][:, ov:ov + 128]
    wp = w_proj[c]
    # row 64 of wp0 carries the bias; stk's ones-row picks it up in the proj
    # matmul's 65-deep contraction.
    wp0 = np.zeros((65, 256), np.float32)
    wp0[0:64, 0:128] = wp[0:64, :]
    wp0[64, 0:128] = b_proj[128 * c:128 * (c + 1)]
    wp1 = np.zeros((65, 256), np.float32)
    wp1[0:64, 0:128] = wp[64:128, :]
    return {
        "xq": xsl(gq),
        "xk": xsl(gk),
        "xv": xsl(gv),
        "wq": np.ascontiguousarray(w_qkv[gq][:, oq:oq + 128]),
        "wk": np.ascontiguousarray(w_qkv[gk][:, ok:ok + 128]),
        "wv": wv,
        "wp0": wp0,
        "wp1": wp1,
    }


def kernel(x, w_qkv, w_proj, b_proj, _trace=False, _trace_kwargs=None):
    x = np.asarray(x, np.float32)
    w_qkv = np.asarray(w_qkv, np.float32)
    w_proj = np.asarray(w_proj, np.float32)
    b_proj = np.asarray(b_proj, np.float32)

    if "nc" not in _CACHE:
        _CACHE["nc"] = _build_nc()
    nc = _CACHE["nc"]

    in_maps = [_core_inputs(x, w_qkv, w_proj, b_proj, c) for c in range(8)]
    res = run_bass_kernel_spmd(
        nc, in_maps, list(range(8)),
        trace=_trace, **(_trace_kwargs or {}),
    )
    out = np.concatenate([res.results[c]["y"] for c in range(8)], axis=2)
    if _trace:
        return out, res
    return out


# revision 17
# speedup vs baseline: 1.1909x; 1.0106x over previous
"""GroupedAttention Trainium2 kernel (8 NeuronCores, SPMD, no collectives).

Problem: x[2,2048,1024] -> grouped qkv (G=8 block-diag) -> 16-head attention
-> grouped proj (G=8 block-diag) + bias.

Sharding: core c owns heads (2c, 2c+1) and proj group c. The proj group c
consumes exactly the attention outputs of heads 2c/2c+1 and produces output
channels [128c, 128c+128) -- so each core computes an independent channel
slice of the final output; outputs are concatenated on the host.

The qkv grouping does NOT align with heads (each qkv group emits a mixed
384-channel slice), so per core we hand it the three 128-channel x-slices
(for its q, k and v blocks) pre-transposed to channel-major [128, B*N],
plus the matching [128(in),128(out)] weight blocks.

Pipeline layout per core:
  phase 1: qT/kT ([128,2048] per batch, rows 0:64 head0 / 64:128 head1) and
    vaug ([128(m), 16(mt), 65] per (b,h); col 64 = ones for the softmax
    denominator) via f32r matmuls; PSUM evacuation split between ACT/DVE.
  phase 2 (per (b, nb) round over 512-query windows): 16 m-tile steps of
    scores (PE) -> exp (ACT, or Schraudolph bit-trick on DVE for a few
    m-tiles to balance engines) -> accumulating AV matmul (PE). Softmax
    denominator rides as row 64 of the AV output. Normalize = reciprocal +
    partition-broadcast (Pool) + multiply into persistent stk.
  phase 3: proj tiles interleaved one round behind, bias-add, per-tile DMA.
"""

import numpy as np
from contextlib import ExitStack

import concourse.bass as bass
import concourse.tile as tile
from concourse import bacc, mybir
from concourse.bass_utils import run_bass_kernel_spmd

F32 = mybir.dt.float32
F32R = mybir.dt.float32r
I32 = mybir.dt.int32
EXP = mybir.ActivationFunctionType.Exp
COPY = mybir.ActivationFunctionType.Copy
MULT = mybir.AluOpType.mult
ADD = mybir.AluOpType.add

B = 2
N = 2048
C = 1024
H = 16
G = 8
D = 64          # head dim
BN = B * N      # 4096
W = 512         # attention n-window per round
NB = N // W     # rounds per batch = 4
MT = N // 128   # m-tiles per batch = 16
SCALE = D ** -0.5

# m-tiles per round whose exp runs on DVE via the Schraudolph bit trick
# (exp(x) ~= bitcast_f32(int32(x*2^23/ln2 + B))), to offload the ACT engine.
DVE_MTS = (6, 9, 12)
SCHRAU_A = SCALE * (2.0 ** 23) / np.log(2.0)          # applied to raw scores
SCHRAU_B = (127.0 - 0.057) * (2.0 ** 23) + 0.5       # centered + trunc->round

_CACHE = {}


def _r(ap):
    return ap if ap.dtype == F32R else ap.bitcast(F32R)


def _build_nc():
    nc = bacc.Bacc("TRN2", target_bir_lowering=False, debug=False, num_devices=8)

    xq = nc.dram_tensor("xq", [128, BN], F32, kind="ExternalInput").ap()
    xk = nc.dram_tensor("xk", [128, BN], F32, kind="ExternalInput").ap()
    xv = nc.dram_tensor("xv", [128, BN], F32, kind="ExternalInput").ap()
    wq = nc.dram_tensor("wq", [128, 128], F32, kind="ExternalInput").ap()
    wk = nc.dram_tensor("wk", [128, 128], F32, kind="ExternalInput").ap()
    wv = nc.dram_tensor("wv", [128, 256], F32, kind="ExternalInput").ap()
    wp0 = nc.dram_tensor("wp0", [65, 256], F32, kind="ExternalInput").ap()
    wp1 = nc.dram_tensor("wp1", [65, 256], F32, kind="ExternalInput").ap()
    y = nc.dram_tensor("y", [B, N, 128], F32, kind="ExternalOutput").ap()

    with ExitStack() as ctx:
        tc = ctx.enter_context(tile.TileContext(nc))
        nc_ = tc.nc

        persist = ctx.enter_context(tc.tile_pool(name="persist", bufs=1))

        # ---- input DMAs: split across SP and Pool queues, ordered by
        # consumption (qk of b0 first, then v of b0, then b1). ----
        wq_t = persist.tile([128, 128], F32R, tag="wq")
        wk_t = persist.tile([128, 128], F32R, tag="wk")
        wv_t = persist.tile([128, 256], F32R, tag="wv")
        wp0_t = persist.tile([65, 256], F32R, tag="wp0")
        wp1_t = persist.tile([65, 256], F32R, tag="wp1")
        xq_t = persist.tile([128, BN], F32R, tag="xq")
        xk_t = persist.tile([128, BN], F32R, tag="xk")
        xv_t = persist.tile([128, BN], F32R, tag="xv")

        # f32 -> f32r casting DMAs must go through the gpsimd (SWDGE) queue.
        def chunk(dst, src, i):
            s = slice(i * 1024, (i + 1) * 1024)
            nc_.gpsimd.dma_start(out=dst[:, s], in_=src[:, s])

        nc_.gpsimd.dma_start(out=wq_t, in_=wq)
        nc_.gpsimd.dma_start(out=wk_t, in_=wk)
        for i in range(2):
            chunk(xq_t, xq, i)
            chunk(xk_t, xk, i)
        nc_.gpsimd.dma_start(out=wv_t, in_=wv)
        for i in range(2):
            chunk(xv_t, xv, i)
        for i in range(2, 4):
            chunk(xq_t, xq, i)
            chunk(xk_t, xk, i)
        for i in range(2, 4):
            chunk(xv_t, xv, i)
        nc_.gpsimd.dma_start(out=wp0_t, in_=wp0)
        nc_.gpsimd.dma_start(out=wp1_t, in_=wp1)

        # ---- persistent activations ----
        qT = [persist.tile([128, N], F32R, tag=f"qT{b}", name=f"qT{b}")
              for b in range(B)]   # rows 0:64 h0, 64:128 h1
        kT = [persist.tile([128, N], F32R, tag=f"kT{b}", name=f"kT{b}")
              for b in range(B)]
        # vaug[b*2+h]: [128(m), MT, 65]; col 64 = ones (softmax denominator)
        vaug = [persist.tile([128, MT, 65], F32R, tag=f"vaug{i}", name=f"vaug{i}")
                for i in range(4)]
        ones_f = persist.tile([128, MT, 1], F32, tag="ones_f")
        nc_.gpsimd.memset(ones_f, 1.0)
        for t in vaug:
            nc_.vector.tensor_copy(out=t[:, :, 64:65], in_=ones_f)
        # normalized attention outputs per (b, h): [65(d), N]; row 64 is
        # ones so the proj matmul's 65-deep contraction adds the bias row
        # carried in wp0.
        stk = [[persist.tile([65, N], F32R, tag=f"stk{b}{h}", name=f"stk{b}{h}")
                for h in range(2)]
               for b in range(B)]
        ones_row = persist.tile([65, N], F32, tag="ones_row")
        nc_.gpsimd.memset(ones_row[64:65, :], 1.0)
        for b in range(B):
            for h in range(2):
                nc_.vector.tensor_copy(out=stk[b][h][64:65, :],
                                       in_=ones_row[64:65, :])

        # ---- phase 2 + 3 pools (phase 1 borrows the stp ring) ----
        stp = ctx.enter_context(tc.tile_pool(name="stp", bufs=3, space="PSUM"))
        avp = ctx.enter_context(tc.tile_pool(name="avp", bufs=2, space="PSUM"))
        ptp = ctx.enter_context(tc.tile_pool(name="ptp", bufs=4))
        nrm = ctx.enter_context(tc.tile_pool(name="nrm", bufs=2))
        outbuf = [persist.tile([128, MT, 128], F32, tag=f"ob{b}", name=f"ob{b}")
                  for b in range(B)]
        # y viewed so one window's 4 n-tiles DMA out in a single transfer
        yr = [y[b, :, :].rearrange("(t p) c -> p t c", p=128) for b in range(B)]

        def ph1_qk(b, i, part, on_act=False):
            # half of one 512-column window of qT/kT for batch b
            s = slice(b * N + i * 512, b * N + (i + 1) * 512)
            sl = slice(i * 512, (i + 1) * 512)
            p = stp.tile([128, 2 * W], F32, tag="st", name=f"p{part}{b}{i}")
            w, dst = (wq_t, qT) if part == "q" else (wk_t, kT)
            src = xq_t if part == "q" else xk_t
            nc_.tensor.matmul(p[:, 0:512], _r(w), _r(src[:, s]),
                              start=True, stop=True)
            if on_act:
                nc_.scalar.activation(out=dst[b][:, sl], in_=p[:, 0:512],
                                      func=COPY)
            else:
                nc_.vector.tensor_copy(out=dst[b][:, sl], in_=p[:, 0:512])

        def ph1_v(b, g, on_act=False):
            # one 128-row m-tile of v for batch b
            s = slice(b * N + g * 128, b * N + (g + 1) * 128)
            pv = stp.tile([128, 2 * W], F32, tag="st", name=f"pv{b}{g}")
            nc_.tensor.matmul(pv[:, 0:256], _r(xv_t[:, s]), _r(wv_t),
                              start=True, stop=True)
            if on_act:
                nc_.scalar.activation(out=vaug[b * 2][:, g, 0:64],
                                      in_=pv[:, 0:64], func=COPY)
            else:
                nc_.vector.tensor_copy(out=vaug[b * 2][:, g, 0:64],
                                       in_=pv[:, 0:64])
            nc_.vector.tensor_copy(out=vaug[b * 2 + 1][:, g, 0:64],
                                   in_=pv[:, 64:128])

        for i in range(4):
            ph1_qk(0, i, "q", on_act=True)
            ph1_qk(0, i, "k")
        for g in range(MT):
            ph1_v(0, g, on_act=True)

        # remaining phase-1 pieces for b=1, interleaved into b=0's rounds
        # (DVE-only copies: ACT is the pacer inside rounds)
        ph1_rest = ([lambda i=i: ph1_qk(1, i, "q") for i in range(4)]
                    + [lambda i=i: ph1_qk(1, i, "k") for i in range(4)]
                    + [lambda g=g: ph1_v(1, g) for g in range(MT)])

        def proj_tile(b, nt):
            s = slice(nt * 128, (nt + 1) * 128)
            pp = stp.tile([128, 2 * W], F32, tag="st", name=f"pp{b}{nt}")
            nc_.tensor.matmul(pp[:, 0:256], _r(stk[b][0][:, s]), _r(wp0_t),
                              start=True, stop=False)
            nc_.tensor.matmul(pp[:, 0:256], _r(stk[b][1][:, s]), _r(wp1_t),
                              start=False, stop=True)
            nc_.vector.tensor_copy(out=outbuf[b][:, nt, :], in_=pp[:, 0:128])
            if nt % 4 == 3:
                nb = nt // 4
                nc_.sync.dma_start(out=yr[b][:, 4 * nb:4 * nb + 4, :],
                                   in_=outbuf[b][:, 4 * nb:4 * nb + 4, :])

        # transient st-ring work (phase-1 b1 pieces, prev round's proj tiles)
        # is emitted right after the scores of designated m-tiles so the ring
        # stays busy without stalling the scores->exp pipeline.
        PIECE_MTS = (1, 2, 3, 4, 5, 7, 8, 10, 11, 13, 14, 15)

        def round_(b, nb, pieces=()):
            n0 = nb * W
            pieces = list(pieces)
            av = [avp.tile([128, W], F32, tag="av", name=f"av{b}{nb}{h}")
                  for h in range(2)]

            def emit_av(mt, pt):
                for h in range(2):
                    nc_.tensor.matmul(
                        av[h][0:65, :],
                        _r(vaug[b * 2 + h][:, mt, :]),
                        _r(pt[:, h * W:(h + 1) * W]),
                        start=(mt == 0), stop=(mt == MT - 1),
                    )

            prev = None
            for mt in range(MT):
                m0 = mt * 128
                st = stp.tile([128, 2 * W], F32, tag="st", name=f"st{b}{nb}{mt}")
                for h in range(2):
                    hs = slice(h * 64, (h + 1) * 64)
                    nc_.tensor.matmul(
                        st[:, h * W:(h + 1) * W],
                        _r(kT[b][hs, m0:m0 + 128]),
                        _r(qT[b][hs, n0:n0 + W]),
                        start=True, stop=True,
                    )
                if prev is not None:
                    emit_av(*prev)
                if pieces and mt in PIECE_MTS:
                    pieces.pop(0)()
                pt = ptp.tile([128, 2 * W], F32R, tag="pt",
                              name=f"pt{b}{nb}{mt}")
                if mt in DVE_MTS:
                    # Schraudolph: int32 bits of exp(s*SCALE), then a
                    # bit-exact same-dtype copy into the f32r tile.
                    pi = ptp.tile([128, 2 * W], I32, tag="pti",
                                  name=f"pi{b}{nb}{mt}", bufs=2)
                    nc_.vector.tensor_scalar(
                        out=pi, in0=st,
                        scalar1=float(SCHRAU_A), scalar2=float(SCHRAU_B),
                        op0=MULT, op1=ADD,
                    )
                    nc_.vector.tensor_copy(out=pt, in_=pi.bitcast(F32R))
                else:
                    nc_.scalar.activation(out=pt, in_=st, func=EXP, scale=SCALE)
                prev = (mt, pt)
            emit_av(*prev)
            for f in pieces:
                f()

            # normalize: stage av out to SBUF first (frees the av slot for
            # the next round), then 1/Z broadcast and multiply, all in SBUF.
            for h in range(2):
                stg = nrm.tile([65, W], F32, tag="stg", name=f"stg{b}{nb}{h}")
                nc_.vector.tensor_copy(out=stg, in_=av[h][0:65, :])
                zr = nrm.tile([1, W], F32, tag="zr", name=f"zr{b}{nb}{h}")
                nc_.vector.tensor_copy(out=zr, in_=stg[64:65, :])
                rz = nrm.tile([1, W], F32, tag="rz", name=f"rz{b}{nb}{h}")
                nc_.vector.reciprocal_approx_fast(out=rz, in_=zr)
                rb = nrm.tile([64, W], F32, tag="rb", name=f"rb{b}{nb}{h}")
                nc_.gpsimd.partition_broadcast(rb, rz)
                nc_.vector.tensor_mul(
                    stk[b][h][0:64, n0:n0 + W], stg[0:64, :], rb)

        rounds = [(b, nb) for b in range(B) for nb in range(NB)]
        for i, (b, nb) in enumerate(rounds):
            pieces = []
            if i >= 1:
                pb, pnb = rounds[i - 1]
                pieces += [lambda nt=nt, pb=pb: proj_tile(pb, nt)
                           for nt in range(pnb * 4, pnb * 4 + 4)]
            if b == 0:
                take = min(len(PIECE_MTS) - len(pieces), len(ph1_rest))
                pieces += ph1_rest[:take]
                ph1_rest = ph1_rest[take:]
            assert len(pieces) <= len(PIECE_MTS)
            round_(b, nb, pieces)
        assert not ph1_rest
        for nt in range(12, 16):
            proj_tile(1, nt)

    nc.finalize()
    return nc


def _core_inputs(x, w_qkv, w_proj, b_proj, c):
    h0 = 2 * c
    gq, oq = divmod(64 * h0, 384)
    gk, ok = divmod(C + 64 * h0, 384)
    gv, ov = divmod(2 * C + 64 * h0, 384)

    def xsl(g):
        # [B,N,128] slice -> channel-major [128, B*N]
        return np.ascontiguousarray(
            x[:, :, 128 * g:128 * (g + 1)].reshape(BN, 128).T
        )

    wv = np.zeros((128, 256), np.float32)
    wv[:, 0:128] = w_qkv[gv][:, ov:ov + 128]
    wp = w_proj[c]
    # row 64 of wp0 carries the bias; stk's ones-row picks it up in the proj
    # matmul's 65-deep contraction.
    wp0 = np.zeros((65, 256), np.float32)
    wp0[0:64, 0:128] = wp[0:64, :]
    wp0[64, 0:128] = b_proj[128 * c:128 * (c + 1)]
    wp1 = np.zeros((65, 256), np.float32)
    wp1[0:64, 0:128] = wp[64:128, :]
    return {
        "xq": xsl(gq),
        "xk": xsl(gk),
        "xv": xsl(gv),
        "wq": np.ascontiguousarray(w_qkv[gq][:, oq:oq + 128]),
        "wk": np.ascontiguousarray(w_qkv[gk][:, ok:ok + 128]),
        "wv": wv,
        "wp0": wp0,
        "wp1": wp1,
    }


def kernel(x, w_qkv, w_proj, b_proj, _trace=False, _trace_kwargs=None):
    x = np.asarray(x, np.float32)
    w_qkv = np.asarray(w_qkv, np.float32)
    w_proj = np.asarray(w_proj, np.float32)
    b_proj = np.asarray(b_proj, np.float32)

    if "nc" not in _CACHE:
        _CACHE["nc"] = _build_nc()
    nc = _CACHE["nc"]

    in_maps = [_core_inputs(x, w_qkv, w_proj, b_proj, c) for c in range(8)]
    res = run_bass_kernel_spmd(
        nc, in_maps, list(range(8)),
        trace=_trace, **(_trace_kwargs or {}),
    )
    out = np.concatenate([res.results[c]["y"] for c in range(8)], axis=2)
    if _trace:
        return out, res
    return out


# revision 18
# speedup vs baseline: 1.2938x; 1.0864x over previous
"""GroupedAttention Trainium2 kernel (8 NeuronCores, SPMD, no collectives).

Problem: x[2,2048,1024] -> grouped qkv (G=8 block-diag) -> 16-head attention
-> grouped proj (G=8 block-diag) + bias.

Sharding: core c owns heads (2c, 2c+1) and proj group c. The proj group c
consumes exactly the attention outputs of heads 2c/2c+1 and produces output
channels [128c, 128c+128) -- so each core computes an independent channel
slice of the final output; outputs are concatenated on the host.

The qkv grouping does NOT align with heads (each qkv group emits a mixed
384-channel slice), so per core we hand it the three 128-channel x-slices
(for its q, k and v blocks) pre-transposed to channel-major [128, B*N],
plus the matching [128(in),128(out)] weight blocks.

Pipeline layout per core:
  phase 1: qT/kT ([128,2048] per batch, rows 0:64 head0 / 64:128 head1) and
    vaug ([128(m), 16(mt), 65] per (b,h); col 64 = ones for the softmax
    denominator) via f32r matmuls; PSUM evacuation split between ACT/DVE.
  phase 2 (per (b, nb) round over 512-query windows): 16 m-tile steps of
    scores (PE) -> exp (ACT, or Schraudolph bit-trick on DVE for a few
    m-tiles to balance engines) -> accumulating AV matmul (PE). Softmax
    denominator rides as row 64 of the AV output. Normalize = reciprocal +
    partition-broadcast (Pool) + multiply into persistent stk.
  phase 3: proj tiles interleaved one round behind, bias-add, per-tile DMA.
"""

import numpy as np
from contextlib import ExitStack

import concourse.bass as bass
import concourse.tile as tile
from concourse import bacc, mybir
from concourse.bass_utils import run_bass_kernel_spmd

F32 = mybir.dt.float32
F32R = mybir.dt.float32r
I32 = mybir.dt.int32
EXP = mybir.ActivationFunctionType.Exp
COPY = mybir.ActivationFunctionType.Copy
MULT = mybir.AluOpType.mult
ADD = mybir.AluOpType.add

B = 2
N = 2048
C = 1024
H = 16
G = 8
D = 64          # head dim
BN = B * N      # 4096
W = 512         # attention n-window per round
NB = N // W     # rounds per batch = 4
MT = N // 128   # m-tiles per batch = 16
SCALE = D ** -0.5

# m-tiles per round whose exp runs on DVE via the Schraudolph bit trick
# (exp(x) ~= bitcast_f32(int32(x*2^23/ln2 + B))), to offload the ACT engine.
DVE_MTS = (6, 9, 12)
SCHRAU_A = SCALE * (2.0 ** 23) / np.log(2.0)          # applied to raw scores
SCHRAU_B = (127.0 - 0.057) * (2.0 ** 23) + 0.5       # centered + trunc->round

_CACHE = {}


def _r(ap):
    return ap if ap.dtype == F32R else ap.bitcast(F32R)


def _build_nc():
    nc = bacc.Bacc("TRN2", target_bir_lowering=False, debug=False, num_devices=8)

    xq = nc.dram_tensor("xq", [128, BN], F32, kind="ExternalInput").ap()
    xk = nc.dram_tensor("xk", [128, BN], F32, kind="ExternalInput").ap()
    xv = nc.dram_tensor("xv", [128, BN], F32, kind="ExternalInput").ap()
    wq = nc.dram_tensor("wq", [128, 128], F32, kind="ExternalInput").ap()
    wk = nc.dram_tensor("wk", [128, 128], F32, kind="ExternalInput").ap()
    wv = nc.dram_tensor("wv", [128, 256], F32, kind="ExternalInput").ap()
    wp0 = nc.dram_tensor("wp0", [65, 256], F32, kind="ExternalInput").ap()
    wp1 = nc.dram_tensor("wp1", [65, 256], F32, kind="ExternalInput").ap()
    y = nc.dram_tensor("y", [B, N, 128], F32, kind="ExternalOutput").ap()

    with ExitStack() as ctx:
        tc = ctx.enter_context(tile.TileContext(nc))
        nc_ = tc.nc

        persist = ctx.enter_context(tc.tile_pool(name="persist", bufs=1))

        # ---- input DMAs: split across SP and Pool queues, ordered by
        # consumption (qk of b0 first, then v of b0, then b1). ----
        wq_t = persist.tile([128, 128], F32R, tag="wq")
        wk_t = persist.tile([128, 128], F32R, tag="wk")
        wv_t = persist.tile([128, 256], F32R, tag="wv")
        wp0_t = persist.tile([65, 256], F32R, tag="wp0")
        wp1_t = persist.tile([65, 256], F32R, tag="wp1")
        xq_t = persist.tile([128, BN], F32R, tag="xq")
        xk_t = persist.tile([128, BN], F32R, tag="xk")
        xv_t = persist.tile([128, BN], F32R, tag="xv")

        # f32 -> f32r casting DMAs must go through the gpsimd (SWDGE) queue.
        def chunk(dst, src, i):
            s = slice(i * 1024, (i + 1) * 1024)
            nc_.gpsimd.dma_start(out=dst[:, s], in_=src[:, s])

        nc_.gpsimd.dma_start(out=wq_t, in_=wq)
        nc_.gpsimd.dma_start(out=wk_t, in_=wk)
        for i in range(2):
            chunk(xq_t, xq, i)
            chunk(xk_t, xk, i)
        nc_.gpsimd.dma_start(out=wv_t, in_=wv)
        for i in range(2):
            chunk(xv_t, xv, i)
        for i in range(2, 4):
            chunk(xq_t, xq, i)
            chunk(xk_t, xk, i)
        for i in range(2, 4):
            chunk(xv_t, xv, i)
        nc_.gpsimd.dma_start(out=wp0_t, in_=wp0)
        nc_.gpsimd.dma_start(out=wp1_t, in_=wp1)

        # ---- persistent activations ----
        qT = [persist.tile([128, N], F32R, tag=f"qT{b}", name=f"qT{b}")
              for b in range(B)]   # rows 0:64 h0, 64:128 h1
        kT = [persist.tile([128, N], F32R, tag=f"kT{b}", name=f"kT{b}")
              for b in range(B)]
        # vaug[b*2+h]: [128(m), MT, 65]; col 64 = ones (softmax denominator)
        vaug = [persist.tile([128, MT, 65], F32R, tag=f"vaug{i}", name=f"vaug{i}")
                for i in range(4)]
        ones_f = persist.tile([128, MT, 1], F32, tag="ones_f")
        nc_.gpsimd.memset(ones_f, 1.0)
        for t in vaug:
            nc_.vector.tensor_copy(out=t[:, :, 64:65], in_=ones_f)
        # normalized attention outputs per (b, h): [65(d), N]; row 64 is
        # ones so the proj matmul's 65-deep contraction adds the bias row
        # carried in wp0.
        stk = [[persist.tile([65, N], F32R, tag=f"stk{b}{h}", name=f"stk{b}{h}")
                for h in range(2)]
               for b in range(B)]
        ones_row = persist.tile([65, N], F32, tag="ones_row")
        nc_.gpsimd.memset(ones_row[64:65, :], 1.0)
        for b in range(B):
            for h in range(2):
                nc_.vector.tensor_copy(out=stk[b][h][64:65, :],
                                       in_=ones_row[64:65, :])

        # ---- phase 2 + 3 pools (phase 1 borrows the stp ring) ----
        stp = ctx.enter_context(tc.tile_pool(name="stp", bufs=3, space="PSUM"))
        avp = ctx.enter_context(tc.tile_pool(name="avp", bufs=2, space="PSUM"))
        ptp = ctx.enter_context(tc.tile_pool(name="ptp", bufs=4))
        nrm = ctx.enter_context(tc.tile_pool(name="nrm", bufs=2))
        outbuf = [persist.tile([128, MT, 128], F32, tag=f"ob{b}", name=f"ob{b}")
                  for b in range(B)]
        # y viewed so one window's 4 n-tiles DMA out in a single transfer
        yr = [y[b, :, :].rearrange("(t p) c -> p t c", p=128) for b in range(B)]

        def ph1_qk(b, i, part, on_act=False):
            # half of one 512-column window of qT/kT for batch b
            s = slice(b * N + i * 512, b * N + (i + 1) * 512)
            sl = slice(i * 512, (i + 1) * 512)
            p = stp.tile([128, 2 * W], F32, tag="st", name=f"p{part}{b}{i}")
            w, dst = (wq_t, qT) if part == "q" else (wk_t, kT)
            src = xq_t if part == "q" else xk_t
            nc_.tensor.matmul(p[:, 0:512], _r(w), _r(src[:, s]),
                              start=True, stop=True)
            if on_act:
                nc_.scalar.activation(out=dst[b][:, sl], in_=p[:, 0:512],
                                      func=COPY)
            else:
                nc_.vector.tensor_copy(out=dst[b][:, sl], in_=p[:, 0:512])

        def ph1_v(b, g, on_act=False):
            # one 128-row m-tile of v for batch b
            s = slice(b * N + g * 128, b * N + (g + 1) * 128)
            pv = stp.tile([128, 2 * W], F32, tag="st", name=f"pv{b}{g}")
            nc_.tensor.matmul(pv[:, 0:256], _r(xv_t[:, s]), _r(wv_t),
                              start=True, stop=True)
            if on_act:
                nc_.scalar.activation(out=vaug[b * 2][:, g, 0:64],
                                      in_=pv[:, 0:64], func=COPY)
            else:
                nc_.vector.tensor_copy(out=vaug[b * 2][:, g, 0:64],
                                       in_=pv[:, 0:64])
            nc_.vector.tensor_copy(out=vaug[b * 2 + 1][:, g, 0:64],
                                   in_=pv[:, 64:128])

        for i in range(4):
            ph1_qk(0, i, "q", on_act=True)
            ph1_qk(0, i, "k")
        for g in range(MT):
            ph1_v(0, g, on_act=True)

        # remaining phase-1 pieces for b=1, interleaved into b=0's rounds
        # (DVE-only copies: ACT is the pacer inside rounds)
        ph1_rest = ([lambda i=i: ph1_qk(1, i, "q") for i in range(4)]
                    + [lambda i=i: ph1_qk(1, i, "k") for i in range(4)]
                    + [lambda g=g: ph1_v(1, g) for g in range(MT)])

        def proj_tile(b, nt):
            s = slice(nt * 128, (nt + 1) * 128)
            pp = stp.tile([128, 2 * W], F32, tag="st", name=f"pp{b}{nt}")
            nc_.tensor.matmul(pp[:, 0:256], _r(stk[b][0][:, s]), _r(wp0_t),
                              start=True, stop=False)
            nc_.tensor.matmul(pp[:, 0:256], _r(stk[b][1][:, s]), _r(wp1_t),
                              start=False, stop=True)
            nc_.vector.tensor_copy(out=outbuf[b][:, nt, :], in_=pp[:, 0:128])
            if nt % 4 == 3:
                nb = nt // 4
                nc_.sync.dma_start(out=yr[b][:, 4 * nb:4 * nb + 4, :],
                                   in_=outbuf[b][:, 4 * nb:4 * nb + 4, :])

        # transient st-ring work (phase-1 b1 pieces, prev round's proj tiles)
        # is emitted right after the scores of designated m-tiles so the ring
        # stays busy without stalling the scores->exp pipeline.
        PIECE_MTS = (1, 2, 3, 4, 5, 7, 8, 10, 11)

        def round_(b, nb, pieces=()):
            n0 = nb * W
            pieces = list(pieces)
            av = [avp.tile([128, W], F32, tag="av", name=f"av{b}{nb}{h}")
                  for h in range(2)]

            def emit_av(mt, pt):
                for h in range(2):
                    nc_.tensor.matmul(
                        av[h][0:65, :],
                        _r(vaug[b * 2 + h][:, mt, :]),
                        _r(pt[:, h * W:(h + 1) * W]),
                        start=(mt == 0), stop=(mt == MT - 1),
                    )

            prev = None
            for mt in range(MT):
                m0 = mt * 128
                st = stp.tile([128, 2 * W], F32, tag="st", name=f"st{b}{nb}{mt}")
                for h in range(2):
                    hs = slice(h * 64, (h + 1) * 64)
                    nc_.tensor.matmul(
                        st[:, h * W:(h + 1) * W],
                        _r(kT[b][hs, m0:m0 + 128]),
                        _r(qT[b][hs, n0:n0 + W]),
                        start=True, stop=True,
                    )
                if prev is not None:
                    emit_av(*prev)
                if pieces and mt in PIECE_MTS:
                    pieces.pop(0)()
                pt = ptp.tile([128, 2 * W], F32R, tag="pt",
                              name=f"pt{b}{nb}{mt}")
                if mt in DVE_MTS:
                    # Schraudolph: int32 bits of exp(s*SCALE), then a
                    # bit-exact same-dtype copy into the f32r tile.
                    pi = ptp.tile([128, 2 * W], I32, tag="pti",
                                  name=f"pi{b}{nb}{mt}", bufs=2)
                    nc_.vector.tensor_scalar(
                        out=pi, in0=st,
                        scalar1=float(SCHRAU_A), scalar2=float(SCHRAU_B),
                        op0=MULT, op1=ADD,
                    )
                    nc_.vector.tensor_copy(out=pt, in_=pi.bitcast(F32R))
                else:
                    nc_.scalar.activation(out=pt, in_=st, func=EXP, scale=SCALE)
                prev = (mt, pt)
            emit_av(*prev)
            for f in pieces:
                f()

            # normalize: stage av out to SBUF first (frees the av slot for
            # the next round), then 1/Z broadcast and multiply, all in SBUF.
            for h in range(2):
                stg = nrm.tile([65, W], F32, tag="stg", name=f"stg{b}{nb}{h}")
                nc_.vector.tensor_copy(out=stg, in_=av[h][0:65, :])
                zr = nrm.tile([1, W], F32, tag="zr", name=f"zr{b}{nb}{h}")
                nc_.vector.tensor_copy(out=zr, in_=stg[64:65, :])
                rz = nrm.tile([1, W], F32, tag="rz", name=f"rz{b}{nb}{h}")
                nc_.vector.reciprocal_approx_fast(out=rz, in_=zr)
                rb = nrm.tile([64, W], F32, tag="rb", name=f"rb{b}{nb}{h}")
                nc_.gpsimd.partition_broadcast(rb, rz)
                nc_.vector.tensor_mul(
                    stk[b][h][0:64, n0:n0 + W], stg[0:64, :], rb)

        rounds = [(b, nb) for b in range(B) for nb in range(NB)]
        for i, (b, nb) in enumerate(rounds):
            pieces = []
            if i >= 2:
                # two-round lag: the source stk windows were normalized a
                # full round ago, so proj never gates on the DVE queue.
                pb, pnb = rounds[i - 2]
                pieces += [lambda nt=nt, pb=pb: proj_tile(pb, nt)
                           for nt in range(pnb * 4, pnb * 4 + 4)]
            if b == 0:
                take = min(len(PIECE_MTS) - len(pieces), len(ph1_rest))
                pieces += ph1_rest[:take]
                ph1_rest = ph1_rest[take:]
            assert len(pieces) <= len(PIECE_MTS)
            round_(b, nb, pieces)
        assert not ph1_rest
        for nt in range(8, 16):
            proj_tile(1, nt)

    nc.finalize()
    return nc


def _core_inputs(x, w_qkv, w_proj, b_proj, c):
    h0 = 2 * c
    gq, oq = divmod(64 * h0, 384)
    gk, ok = divmod(C + 64 * h0, 384)
    gv, ov = divmod(2 * C + 64 * h0, 384)

    def xsl(g):
        # [B,N,128] slice -> channel-major [128, B*N]
        return np.ascontiguousarray(
            x[:, :, 128 * g:128 * (g + 1)].reshape(BN, 128).T
        )

    wv = np.zeros((128, 256), np.float32)
    wv[:, 0:128] = w_qkv[gv][:, ov:ov + 128]
    wp = w_proj[c]
    # row 64 of wp0 carries the bias; stk's ones-row picks it up in the proj
    # matmul's 65-deep contraction.
    wp0 = np.zeros((65, 256), np.float32)
    wp0[0:64, 0:128] = wp[0:64, :]
    wp0[64, 0:128] = b_proj[128 * c:128 * (c + 1)]
    wp1 = np.zeros((65, 256), np.float32)
    wp1[0:64, 0:128] = wp[64:128, :]
    return {
        "xq": xsl(gq),
        "xk": xsl(gk),
        "xv": xsl(gv),
        "wq": np.ascontiguousarray(w_qkv[gq][:, oq:oq + 128]),
        "wk": np.ascontiguousarray(w_qkv[gk][:, ok:ok + 128]),
        "wv": wv,
        "wp0": wp0,
        "wp1": wp1,
    }


def kernel(x, w_qkv, w_proj, b_proj, _trace=False, _trace_kwargs=None):
    x = np.asarray(x, np.float32)
    w_qkv = np.asarray(w_qkv, np.float32)
    w_proj = np.asarray(w_proj, np.float32)
    b_proj = np.asarray(b_proj, np.float32)

    if "nc" not in _CACHE:
        _CACHE["nc"] = _build_nc()
    nc = _CACHE["nc"]

    in_maps = [_core_inputs(x, w_qkv, w_proj, b_proj, c) for c in range(8)]
    res = run_bass_kernel_spmd(
        nc, in_maps, list(range(8)),
        trace=_trace, **(_trace_kwargs or {}),
    )
    out = np.concatenate([res.results[c]["y"] for c in range(8)], axis=2)
    if _trace:
        return out, res
    return out


# revision 20
# speedup vs baseline: 1.3311x; 1.0288x over previous
"""GroupedAttention Trainium2 kernel (8 NeuronCores, SPMD, no collectives).

Problem: x[2,2048,1024] -> grouped qkv (G=8 block-diag) -> 16-head attention
-> grouped proj (G=8 block-diag) + bias.

Sharding: core c owns heads (2c, 2c+1) and proj group c. The proj group c
consumes exactly the attention outputs of heads 2c/2c+1 and produces output
channels [128c, 128c+128) -- so each core computes an independent channel
slice of the final output; outputs are concatenated on the host.

The qkv grouping does NOT align with heads (each qkv group emits a mixed
384-channel slice), so per core we hand it the three 128-channel x-slices
(for its q, k and v blocks) pre-transposed to channel-major [128, B*N],
plus the matching [128(in),128(out)] weight blocks.

Pipeline layout per core:
  phase 1: qT/kT ([128,2048] per batch, rows 0:64 head0 / 64:128 head1) and
    vaug ([128(m), 16(mt), 65] per (b,h); col 64 = ones for the softmax
    denominator) via f32r matmuls; PSUM evacuation split between ACT/DVE.
  phase 2 (per (b, nb) round over 512-query windows): 16 m-tile steps of
    scores (PE) -> exp (ACT, or Schraudolph bit-trick on DVE for a few
    m-tiles to balance engines) -> accumulating AV matmul (PE). Softmax
    denominator rides as row 64 of the AV output. Normalize = reciprocal +
    partition-broadcast (Pool) + multiply into persistent stk.
  phase 3: proj tiles interleaved one round behind, bias-add, per-tile DMA.
"""

import numpy as np
from contextlib import ExitStack

import concourse.bass as bass
import concourse.tile as tile
from concourse import bacc, mybir
from concourse.bass_utils import run_bass_kernel_spmd

F32 = mybir.dt.float32
F32R = mybir.dt.float32r
I32 = mybir.dt.int32
EXP = mybir.ActivationFunctionType.Exp
COPY = mybir.ActivationFunctionType.Copy
MULT = mybir.AluOpType.mult
ADD = mybir.AluOpType.add

B = 2
N = 2048
C = 1024
H = 16
G = 8
D = 64          # head dim
BN = B * N      # 4096
W = 512         # attention n-window per round
NB = N // W     # rounds per batch = 4
MT = N // 128   # m-tiles per batch = 16
SCALE = D ** -0.5

# m-tiles per round whose exp runs on DVE via the Schraudolph bit trick
# (exp(x) ~= bitcast_f32(int32(x*2^23/ln2 + B))), to offload the ACT engine.
DVE_MTS = (6, 9, 12)
SCHRAU_A = SCALE * (2.0 ** 23) / np.log(2.0)          # applied to raw scores
SCHRAU_B = (127.0 - 0.057) * (2.0 ** 23) + 0.5       # centered + trunc->round

_CACHE = {}


def _r(ap):
    return ap if ap.dtype == F32R else ap.bitcast(F32R)


def _build_nc():
    nc = bacc.Bacc("TRN2", target_bir_lowering=False, debug=False, num_devices=8)

    xq = nc.dram_tensor("xq", [128, BN], F32R, kind="ExternalInput").ap()
    xk = nc.dram_tensor("xk", [128, BN], F32R, kind="ExternalInput").ap()
    xv = nc.dram_tensor("xv", [128, BN], F32R, kind="ExternalInput").ap()
    wq = nc.dram_tensor("wq", [128, 128], F32R, kind="ExternalInput").ap()
    wk = nc.dram_tensor("wk", [128, 128], F32R, kind="ExternalInput").ap()
    wv = nc.dram_tensor("wv", [128, 256], F32R, kind="ExternalInput").ap()
    wpc = nc.dram_tensor("wpc", [128, 256], F32R, kind="ExternalInput").ap()
    bias = nc.dram_tensor("bias", [128, 128], F32, kind="ExternalInput").ap()
    y = nc.dram_tensor("y", [B, N, 128], F32, kind="ExternalOutput").ap()

    with ExitStack() as ctx:
        tc = ctx.enter_context(tile.TileContext(nc))
        nc_ = tc.nc

        persist = ctx.enter_context(tc.tile_pool(name="persist", bufs=1))

        # ---- input DMAs: split across SP and Pool queues, ordered by
        # consumption (qk of b0 first, then v of b0, then b1). ----
        wq_t = persist.tile([128, 128], F32R, tag="wq")
        wk_t = persist.tile([128, 128], F32R, tag="wk")
        wv_t = persist.tile([128, 256], F32R, tag="wv")
        wpc_t = persist.tile([128, 256], F32R, tag="wpc")
        bias_t = persist.tile([128, 128], F32, tag="bias")
        xq_t = persist.tile([128, BN], F32R, tag="xq")
        xk_t = persist.tile([128, BN], F32R, tag="xk")
        xv_t = persist.tile([128, BN], F32R, tag="xv")

        # same-dtype f32r DMAs, spread over three otherwise-idle queues so
        # descriptor-generation does not serialize; first chunks are small so
        # phase 1 starts early.
        CHUNKS = [(0, 512), (512, 512), (1024, 1024), (2048, 1024), (3072, 1024)]

        def chunk(eng, dst, src, i):
            o, ln = CHUNKS[i]
            eng.dma_start(out=dst[:, o:o + ln], in_=src[:, o:o + ln])

        nc_.sync.dma_start(out=wq_t, in_=wq)
        nc_.scalar.dma_start(out=wk_t, in_=wk)
        nc_.gpsimd.dma_start(out=wv_t, in_=wv)
        for i in range(2):
            chunk(nc_.sync, xq_t, xq, i)
            chunk(nc_.scalar, xk_t, xk, i)
            chunk(nc_.gpsimd, xv_t, xv, i)
        for i in range(2, 5):
            chunk(nc_.sync, xq_t, xq, i)
            chunk(nc_.scalar, xk_t, xk, i)
            chunk(nc_.gpsimd, xv_t, xv, i)
        nc_.sync.dma_start(out=wpc_t, in_=wpc)
        nc_.sync.dma_start(out=bias_t, in_=bias)

        # ---- persistent activations ----
        qT = [persist.tile([128, N], F32R, tag=f"qT{b}", name=f"qT{b}")
              for b in range(B)]   # rows 0:64 h0, 64:128 h1
        kT = [persist.tile([128, N], F32R, tag=f"kT{b}", name=f"kT{b}")
              for b in range(B)]
        # vaug[b*2+h]: [128(m), MT, 65]; col 64 = ones (softmax denominator)
        vaug = [persist.tile([128, MT, 65], F32R, tag=f"vaug{i}", name=f"vaug{i}")
                for i in range(4)]
        ones_f = persist.tile([128, MT, 1], F32, tag="ones_f")
        nc_.gpsimd.memset(ones_f, 1.0)
        for t in vaug:
            nc_.vector.tensor_copy(out=t[:, :, 64:65], in_=ones_f)
        # normalized attention outputs per b: [128(d of both heads), N]
        stk = [persist.tile([128, N], F32R, tag=f"stk{b}", name=f"stk{b}")
               for b in range(B)]

        # ---- phase 2 + 3 pools (phase 1 borrows the stp ring) ----
        stp = ctx.enter_context(tc.tile_pool(name="stp", bufs=3, space="PSUM"))
        avp = ctx.enter_context(tc.tile_pool(name="avp", bufs=2, space="PSUM"))
        ptp = ctx.enter_context(tc.tile_pool(name="ptp", bufs=4))
        nrm = ctx.enter_context(tc.tile_pool(name="nrm", bufs=2))
        outbuf = [persist.tile([128, MT, 128], F32, tag=f"ob{b}", name=f"ob{b}")
                  for b in range(B)]
        # y viewed so one window's 4 n-tiles DMA out in a single transfer
        yr = [y[b, :, :].rearrange("(t p) c -> p t c", p=128) for b in range(B)]

        def ph1_qk(b, i, part, on_act=False):
            # half of one 512-column window of qT/kT for batch b
            s = slice(b * N + i * 512, b * N + (i + 1) * 512)
            sl = slice(i * 512, (i + 1) * 512)
            p = stp.tile([128, 2 * W], F32, tag="st", name=f"p{part}{b}{i}")
            w, dst = (wq_t, qT) if part == "q" else (wk_t, kT)
            src = xq_t if part == "q" else xk_t
            nc_.tensor.matmul(p[:, 0:512], _r(w), _r(src[:, s]),
                              start=True, stop=True)
            if on_act:
                nc_.scalar.activation(out=dst[b][:, sl], in_=p[:, 0:512],
                                      func=COPY)
            else:
                nc_.vector.tensor_copy(out=dst[b][:, sl], in_=p[:, 0:512])

        def ph1_v(b, g, on_act=False):
            # one 128-row m-tile of v for batch b
            s = slice(b * N + g * 128, b * N + (g + 1) * 128)
            pv = stp.tile([128, 2 * W], F32, tag="st", name=f"pv{b}{g}")
            nc_.tensor.matmul(pv[:, 0:256], _r(xv_t[:, s]), _r(wv_t),
                              start=True, stop=True)
            if on_act:
                nc_.scalar.activation(out=vaug[b * 2][:, g, 0:64],
                                      in_=pv[:, 0:64], func=COPY)
            else:
                nc_.vector.tensor_copy(out=vaug[b * 2][:, g, 0:64],
                                       in_=pv[:, 0:64])
            nc_.vector.tensor_copy(out=vaug[b * 2 + 1][:, g, 0:64],
                                   in_=pv[:, 64:128])

        for i in range(4):
            ph1_qk(0, i, "q", on_act=True)
            ph1_qk(0, i, "k")
        for g in range(MT):
            ph1_v(0, g, on_act=True)

        # remaining phase-1 pieces for b=1, interleaved into b=0's rounds
        # (DVE-only copies: ACT is the pacer inside rounds)
        ph1_rest = ([lambda i=i: ph1_qk(1, i, "q") for i in range(4)]
                    + [lambda i=i: ph1_qk(1, i, "k") for i in range(4)]
                    + [lambda g=g: ph1_v(1, g) for g in range(MT)])

        def proj_tile(b, nt):
            s = slice(nt * 128, (nt + 1) * 128)
            pp = stp.tile([128, 2 * W], F32, tag="st", name=f"pp{b}{nt}")
            nc_.tensor.matmul(pp[:, 0:256], _r(stk[b][:, s]), _r(wpc_t),
                              start=True, stop=True)
            nc_.vector.tensor_add(outbuf[b][:, nt, :], pp[:, 0:128], bias_t)
            if nt % 4 == 3:
                nb = nt // 4
                nc_.sync.dma_start(out=yr[b][:, 4 * nb:4 * nb + 4, :],
                                   in_=outbuf[b][:, 4 * nb:4 * nb + 4, :])

        # transient st-ring work (phase-1 b1 pieces, prev round's proj tiles)
        # is emitted right after the scores of designated m-tiles so the ring
        # stays busy without stalling the scores->exp pipeline.
        PIECE_MTS = (1, 2, 3, 4, 5, 7, 8, 10, 11)

        def round_(b, nb, pieces=()):
            n0 = nb * W
            pieces = list(pieces)
            av = [avp.tile([128, W], F32, tag="av", name=f"av{b}{nb}{h}")
                  for h in range(2)]

            def emit_av(mt, pt):
                for h in range(2):
                    nc_.tensor.matmul(
                        av[h][0:65, :],
                        _r(vaug[b * 2 + h][:, mt, :]),
                        _r(pt[:, h * W:(h + 1) * W]),
                        start=(mt == 0), stop=(mt == MT - 1),
                    )

            prev = None
            for mt in range(MT):
                m0 = mt * 128
                st = stp.tile([128, 2 * W], F32, tag="st", name=f"st{b}{nb}{mt}")
                for h in range(2):
                    hs = slice(h * 64, (h + 1) * 64)
                    nc_.tensor.matmul(
                        st[:, h * W:(h + 1) * W],
                        _r(kT[b][hs, m0:m0 + 128]),
                        _r(qT[b][hs, n0:n0 + W]),
                        start=True, stop=True,
                    )
                if prev is not None:
                    emit_av(*prev)
                if pieces and mt in PIECE_MTS:
                    pieces.pop(0)()
                pt = ptp.tile([128, 2 * W], F32R, tag="pt",
                              name=f"pt{b}{nb}{mt}")
                if mt in DVE_MTS:
                    # Schraudolph: int32 bits of exp(s*SCALE), then a
                    # bit-exact same-dtype copy into the f32r tile.
                    pi = ptp.tile([128, 2 * W], I32, tag="pti",
                                  name=f"pi{b}{nb}{mt}", bufs=2)
                    nc_.vector.tensor_scalar(
                        out=pi, in0=st,
                        scalar1=float(SCHRAU_A), scalar2=float(SCHRAU_B),
                        op0=MULT, op1=ADD,
                    )
                    nc_.vector.tensor_copy(out=pt, in_=pi.bitcast(F32R))
                else:
                    nc_.scalar.activation(out=pt, in_=st, func=EXP, scale=SCALE)
                prev = (mt, pt)
            emit_av(*prev)
            for f in pieces:
                f()

            # normalize: stage av out to SBUF first (frees the av slot for
            # the next round), then 1/Z broadcast and multiply, all in SBUF.
            for h in range(2):
                stg = nrm.tile([65, W], F32, tag="stg", name=f"stg{b}{nb}{h}")
                nc_.vector.tensor_copy(out=stg, in_=av[h][0:65, :])
                zr = nrm.tile([1, W], F32, tag="zr", name=f"zr{b}{nb}{h}")
                nc_.vector.tensor_copy(out=zr, in_=stg[64:65, :])
                rz = nrm.tile([1, W], F32, tag="rz", name=f"rz{b}{nb}{h}")
                nc_.vector.reciprocal_approx_fast(out=rz, in_=zr)
                rb = nrm.tile([64, W], F32, tag="rb", name=f"rb{b}{nb}{h}")
                nc_.gpsimd.partition_broadcast(rb, rz)
                nc_.vector.tensor_mul(
                    stk[b][h * 64:(h + 1) * 64, n0:n0 + W], stg[0:64, :], rb)

        rounds = [(b, nb) for b in range(B) for nb in range(NB)]
        for i, (b, nb) in enumerate(rounds):
            pieces = []
            if i >= 2:
                # two-round lag: the source stk windows were normalized a
                # full round ago, so proj never gates on the DVE queue.
                pb, pnb = rounds[i - 2]
                pieces += [lambda nt=nt, pb=pb: proj_tile(pb, nt)
                           for nt in range(pnb * 4, pnb * 4 + 4)]
            if b == 0:
                take = min(len(PIECE_MTS) - len(pieces), len(ph1_rest))
                pieces += ph1_rest[:take]
                ph1_rest = ph1_rest[take:]
            assert len(pieces) <= len(PIECE_MTS)
            round_(b, nb, pieces)
        assert not ph1_rest
        for nt in range(8, 16):
            proj_tile(1, nt)

    nc.finalize()
    return nc


def _core_inputs(x, w_qkv, w_proj, b_proj, c):
    h0 = 2 * c
    gq, oq = divmod(64 * h0, 384)
    gk, ok = divmod(C + 64 * h0, 384)
    gv, ov = divmod(2 * C + 64 * h0, 384)

    def xsl(g):
        # [B,N,128] slice -> channel-major [128, B*N]
        return np.ascontiguousarray(
            x[:, :, 128 * g:128 * (g + 1)].reshape(BN, 128).T
        )

    wv = np.zeros((128, 256), np.float32)
    wv[:, 0:128] = w_qkv[gv][:, ov:ov + 128]
    wpc = np.zeros((128, 256), np.float32)
    wpc[:, 0:128] = w_proj[c]
    return {
        "xq": xsl(gq),
        "xk": xsl(gk),
        "xv": xsl(gv),
        "wq": np.ascontiguousarray(w_qkv[gq][:, oq:oq + 128]),
        "wk": np.ascontiguousarray(w_qkv[gk][:, ok:ok + 128]),
        "wv": wv,
        "wpc": wpc,
        "bias": np.ascontiguousarray(
            np.broadcast_to(b_proj[128 * c:128 * (c + 1)], (128, 128))
        ).astype(np.float32),
    }


def kernel(x, w_qkv, w_proj, b_proj, _trace=False, _trace_kwargs=None):
    x = np.asarray(x, np.float32)
    w_qkv = np.asarray(w_qkv, np.float32)
    w_proj = np.asarray(w_proj, np.float32)
    b_proj = np.asarray(b_proj, np.float32)

    if "nc" not in _CACHE:
        _CACHE["nc"] = _build_nc()
    nc = _CACHE["nc"]

    in_maps = [_core_inputs(x, w_qkv, w_proj, b_proj, c) for c in range(8)]
    res = run_bass_kernel_spmd(
        nc, in_maps, list(range(8)),
        trace=_trace, **(_trace_kwargs or {}),
    )
    out = np.concatenate([res.results[c]["y"] for c in range(8)], axis=2)
    if _trace:
        return out, res
    return out
